# revision 35
# baseline (speedup 1.0000x reference)
"""GraphWaveNet kernel for Trainium2 (Bass/Tile), 8 NeuronCores.

Design: edge sharding by SOURCE block, dense slot packing, and TWO
independent slice-pair pipelines (A = batch slices 0,1; B = 2,3) so the
four per-pipeline ReduceScatters interleave with each other's compute.

- Only t=11 survives the final 1x1 conv and the GCN doesn't mix time, so
  the conv stack is evaluated at t in {10,11} only and the GCN runs on
  B=4 slices packed as 256 columns (4 slices x 64 feats); pipeline A
  owns cols 0:128, B owns 128:256. The slices never mix, so A and B are
  fully independent after the conv -- their collectives pipeline:
    g0 A0 B0 RS0A [RS0B | fin0A g1a A1] RS1A [fin0B g1b B1] RS1B [fin1A] fin1B
- GCN identity: with Hs = dsq*h, agg_n = dsq_n * (sum_{e->n} Hs[src_e]
  + Hs[n]), then @W + b + relu.
- Sharding: core k owns node rows [1280k, 1280(k+1)). Edges live on the
  core owning their SRC, so gathers (dma_gather) read only the local
  table. Dense slot packing: block b's edges sit at global positions
  [S[b], S[b]+maxcnt[b]) where maxcnt = max over cores (SPMD-uniform
  schedule); specs are the 128-boundary pieces, each one one-hot P
  matmul into the block's PSUM accumulator (5-block group tiles, copy +
  DMA per group) forming a bf16 partial aggregate over all 10240 rows.
  One ReduceScatter per (layer, pipeline) returns each core its rows.
- PSUM chains: specs are emitted in position (= block-major) order, so
  a chain closes before the next block's opens -- never two open chains
  in one bank.
- Layer 0 gathers the full 256-col table once; both pipelines' scatter
  matmuls slice the same gathered tiles. Layer 1 tables are per-pipeline
  (128 cols), gathered separately with the same slot positions (eidx1).
- HBM layouts are chosen for >=512B contiguous DMA runs: aggN group
  regions are (p-major, q-minor) so each partition writes 1280B; table1
  rows are permuted (per 256-node bp: row = p*2 + lb) so finish writes
  512B runs; eidx1 bakes the permutation into the gather indices.
- dma_gather ucode contract (queue 0): flat index j of an instruction
  lands at out[j%128, j//128] and is read from idx tile position
  [16 + j%16, j//16] (int16).
"""

import sys

sys.path.insert(0, "/opt/trn_rl_repo")

import numpy as np
import ml_dtypes

B, T, N, FIN, H, E = 4, 12, 10000, 2, 64, 80000
NCORES = 8
NB80 = 80                 # dst blocks of 128 nodes
NP = NB80 * 128           # padded node count (10240)
NSH = NP // NCORES        # node rows per core (1280)
NBC = NB80 // NCORES      # node blocks per core (10)
D = 4 * H                 # 256 = 4 slices x 64 feats
DH = D // 2               # 128 = one pipeline's cols
SPG = 8                   # slots (of 128 edges) per dma_gather (1024 idxs)
NIG = SPG * 128           # indices per full gather instruction
ICPG = NIG // 16          # idx tile columns per full gather instr (64)

_cache = {}


def _balance(src, dst):
    """Node -> row permutation so every (core, dst-block) edge count <= 128.
    Phase 1: swap nodes between cores until pairwise core->core edge counts
    fit 10 blocks x 128. Phase 2: per-core greedy min-max packing into 10
    blocks with a repair pass. Heuristic: any residual overflow is handled
    by the dense spec packing (extra matmul pieces), not a correctness issue.
    """
    NPC = N // NCORES + (1 if N % NCORES else 0)   # 1250
    kc = np.minimum(np.arange(N) // NPC, NCORES - 1)
    out_deg = np.bincount(src, minlength=N)

    def mkT(kc):
        T = np.zeros((NCORES, NCORES), np.int64)
        np.add.at(T, (kc[src], kc[dst]), 1)
        return T

    T = mkT(kc)
    for _ in range(400):
        k, c = np.unravel_index(np.argmax(T), T.shape)
        if T[k, c] <= 1270:
            break
        ink = np.bincount(dst[kc[src] == k], minlength=N)
        nodes_c = np.flatnonzero(kc == c)
        n = nodes_c[np.argmax(ink[nodes_c] - 0.1 * out_deg[nodes_c])]
        c2 = np.argmin(T[k] + np.where(np.arange(NCORES) == c, 10 ** 9, 0))
        nodes_c2 = np.flatnonzero(kc == c2)
        n2 = nodes_c2[np.argmin(ink[nodes_c2] + 0.1 * out_deg[nodes_c2])]
        kc[n], kc[n2] = c2, c
        T = mkT(kc)

    dvec = np.zeros((N, NCORES), np.int64)
    np.add.at(dvec, (dst, kc[src]), 1)
    row_of_node = np.full(N, -1, np.int64)
    for c in range(NCORES):
        nodes = np.flatnonzero(kc == c)
        vv = dvec[nodes]
        order = np.argsort(-vv.sum(1), kind="stable")
        load = np.zeros((NBC, NCORES), np.int64)
        nn = np.zeros(NBC, np.int64)
        assign = np.zeros(len(nodes), np.int64)
        for i in order:
            v = vv[i]
            cand = np.flatnonzero(nn < 128)
            newmax = (load[cand] + v).max(axis=1)
            ok = newmax <= 128
            if ok.any():
                c2_ = cand[ok]
                b = c2_[np.argmin((load[c2_] + v).max(axis=1) * 1000
                                  + nn[c2_])]
            else:
                b = cand[np.argmin(newmax)]
            assign[i] = b
            load[b] += v
            nn[b] += 1
        for _ in range(300):
            viol = np.argwhere(load > 128)
            if len(viol) == 0:
                break
            b, k = viol[np.argmax(load[viol[:, 0], viol[:, 1]])]
            members = np.flatnonzero(assign == b)
            cand_n = members[vv[members, k] > 0]
            cand_n = cand_n[np.argsort(-vv[cand_n, k])]
            done = False
            for i in cand_n[:20]:
                v = vv[i]
                tgt = np.flatnonzero((nn < 128)
                                     & ((load + v) <= 128).all(axis=1))
                tgt = tgt[tgt != b]
                if len(tgt):
                    t = tgt[np.argmin((load[tgt] + v).max(axis=1))]
                    assign[i] = t
                    load[b] -= v
                    load[t] += v
                    nn[b] -= 1
                    nn[t] += 1
                    done = True
                    break
            if not done:
                break
        for b in range(NBC):
            sel = nodes[assign == b]
            base = c * NSH + b * 128
            row_of_node[sel] = base + np.arange(len(sel))
    return row_of_node


def _host_prep(x, edge_index, w1, b1, w2, b2, gw1, gb1, gw2, gb2, wo, bo):
    x = np.asarray(x, np.float32)
    src0 = np.asarray(edge_index[0]).astype(np.int64)
    dst0 = np.asarray(edge_index[1]).astype(np.int64)
    rown = _balance(src0, dst0)
    src, dst = rown[src0], rown[dst0]

    deg = np.bincount(dst0, minlength=N).astype(np.float64) + 1.0
    dsq = (deg ** -0.5).astype(np.float32)
    dsq_pad = np.ones(NP, dtype=np.float32)
    dsq_pad[rown] = dsq

    # ---- per-core edge partition by src owner, dst-sorted
    owner = src // NSH
    es_k, ed_k, cnt = [], [], np.zeros((NCORES, NB80), np.int64)
    for k in range(NCORES):
        m = owner == k
        es, ed = src[m], dst[m]
        o = np.argsort(ed, kind="stable")
        es_k.append(es[o])
        ed_k.append(ed[o])
        cnt[k] = np.bincount(ed[o] // 128, minlength=NB80)

    # dense schedule: block b's edges at positions [S[b], S[b]+mc[b])
    mc = np.maximum(1, cnt.max(axis=0))          # SPMD-uniform per block
    S = np.zeros(NB80 + 1, np.int64)
    S[1:] = np.cumsum(mc)
    TOTE = int(S[NB80])
    TOT = (TOTE + 127) // 128                    # slots
    # specs: 128-boundary pieces of each block segment, in position order
    specs = []                                   # (block, slot, pbase, plen, first, last)
    for b in range(NB80):
        a, e = int(S[b]), int(S[b] + mc[b])
        p = a
        while p < e:
            q = min(e, (p // 128 + 1) * 128)
            specs.append((b, p // 128, p % 128, q - p, p == a, q == e))
            p = q
    NMM = len(specs)

    # eidx / eidx1 / P (per-core data; schedule above is uniform)
    NGI = (TOT + SPG - 1) // SPG
    COLS = NGI * ICPG
    eidx_all = np.zeros((NCORES, 128, COLS), np.int16)
    eidx1_all = np.zeros((NCORES, 128, COLS), np.int16)
    P_all = np.zeros((NCORES, 128, NMM * 128), np.float32)
    for k in range(NCORES):
        es, ed = es_k[k], ed_k[k]
        bounds = np.searchsorted(ed, np.arange(NB80 + 1) * 128)
        for b in range(NB80):
            e0, e1 = int(bounds[b]), int(bounds[b + 1])
            ne = e1 - e0
            if ne == 0:
                continue
            jj = int(S[b]) + np.arange(ne)       # global positions
            rows = 16 + (jj % 16)
            cols = (jj // NIG) * ICPG + (jj % NIG) // 16
            loc = (es[e0:e1] - k * NSH).astype(np.int64)
            eidx_all[k, rows, cols] = loc.astype(np.int16)
            # table1 physical row: per 256-node bp, row = p*2 + lb
            lbg, p = loc // 128, loc % 128
            phys = (lbg // 2) * 256 + p * 2 + (lbg % 2)
            eidx1_all[k, rows, cols] = phys.astype(np.int16)
        for mmi, (b, slot, pbase, plen, first, last) in enumerate(specs):
            gpos = slot * 128 + pbase            # global position of spec start
            e0 = int(bounds[b]) + (gpos - int(S[b]))
            ne = min(plen, int(bounds[b + 1]) - e0)
            if ne > 0:
                P_all[k, pbase + np.arange(ne),
                      mmi * 128 + (ed[e0:e0 + ne] - b * 128)] = 1.0
    P_all = P_all.astype(ml_dtypes.float8_e4m3fn)   # one-hot: 1.0 exact in fp8

    # ---- conv input: per block 8 rows (t,c) for t in {9,10,11} + 2 zero rows,
    # cols = 4 slices x 128 nodes
    xpad = np.zeros((B, 3, FIN, NP), np.float32)
    xpad[:, :, :, rown] = x[:, 9:12, :, :].transpose(0, 1, 3, 2)  # [s, ti, c, n]
    xv = xpad.reshape(B, 6, NCORES, NBC, 128)                   # [s, row, k, blk, p]
    xt_all = np.zeros((NCORES, 8, NBC * 4 * 128), np.float32)
    xt_all[:, :6] = xv.transpose(2, 1, 3, 0, 4).reshape(NCORES, 6, NBC * 4 * 128)
    xt_all = xt_all.astype(ml_dtypes.bfloat16)

    dsqk_all = dsq_pad.reshape(NCORES, NBC, 128).transpose(0, 2, 1).copy()

    # ---- weights
    W1m = np.zeros((6, 64), np.float32)
    for kk in range(3):
        for c in range(FIN):
            W1m[2 * kk + c, :] = w1[:, c, 0, kk]
    W1ab = np.zeros((8, 128), np.float32)
    W1ab[0:6, 0:64] = W1m          # A: t10 (taps t9,t10,t11)
    W1ab[2:8, 64:128] = W1m        # B: t11 (taps t10,t11,t12=pad)
    W1ab = W1ab.astype(ml_dtypes.bfloat16)

    W2m = np.zeros((128, 64), np.float32)
    W2m[:64, :] = w2[:, :, 0, 0].T
    W2m[64:, :] = w2[:, :, 0, 1].T
    W2m = W2m.astype(ml_dtypes.bfloat16)

    b1s = np.concatenate([b1, b1]).reshape(128, 1).astype(np.float32)
    b2c = np.asarray(b2, np.float32).reshape(64, 1)
    gb1s = np.concatenate([gb1, gb1]).reshape(128, 1).astype(np.float32)
    gb2s = np.concatenate([gb2, gb2]).reshape(128, 1).astype(np.float32)
    gwd1 = np.zeros((128, 128), np.float32)
    gwd1[0:64, 0:64] = gw1
    gwd1[64:128, 64:128] = gw1
    gwd1 = gwd1.astype(ml_dtypes.bfloat16)
    gwd2 = np.zeros((128, 128), np.float32)
    gwd2[0:64, 0:64] = gw2
    gwd2[64:128, 64:128] = gw2
    gwd2 = gwd2.astype(ml_dtypes.bfloat16)
    wov = np.asarray(wo, np.float32)[0, :, 0, 0]
    wod2 = np.zeros((128, 2), np.float32)
    wod2[0:64, 0] = wov
    wod2[64:128, 1] = wov
    wod2 = wod2.astype(ml_dtypes.bfloat16)

    # pack all small weights into two tensors (one DMA each)
    cwb = np.zeros((128, 450), ml_dtypes.bfloat16)
    cwb[0:8, 0:128] = W1ab
    cwb[:, 128:192] = W2m
    cwb[:, 192:320] = gwd1
    cwb[:, 320:448] = gwd2
    cwb[:, 448:450] = wod2
    cwf = np.zeros((128, 14), np.float32)
    cwf[:, 0:1] = b1s
    cwf[0:64, 1:2] = b2c
    cwf[:, 2:3] = gb1s
    cwf[:, 3:4] = gb2s

    shared = {"cwb": cwb, "cwf": cwf}
    in_maps = []
    for k in range(NCORES):
        m = dict(shared)
        cf = m["cwf"].copy()
        cf[:, 4:14] = dsqk_all[k]
        m["cwf"] = cf
        m["xt"] = xt_all[k]
        m["eidx"] = eidx_all[k]
        m["eidx1"] = eidx1_all[k]
        m["P"] = P_all[k]
        in_maps.append(m)
    return (in_maps, tuple(specs), TOT,
            float(np.asarray(bo).reshape(-1)[0]), rown)


def _build(specs, TOT, bo_f):
    from concourse import bass, bacc, tile
    from concourse.masks import make_identity
    import mybir

    f32, bf16, i16 = mybir.dt.float32, mybir.dt.bfloat16, mybir.dt.int16
    NMM = len(specs)
    NGI = (TOT + SPG - 1) // SPG
    COLS = NGI * ICPG

    nc = bacc.Bacc("TRN2", target_bir_lowering=False, debug=False, num_devices=8,
                   dynamic_dma_scratch_size=65536)

    ext = {}
    for name, shape, dt in [
        ("xt", [8, NBC * 512], bf16), ("cwb", [128, 450], bf16),
        ("cwf", [128, 14], f32),
        ("eidx", [128, COLS], i16), ("eidx1", [128, COLS], i16),
        ("P", [128, NMM * 128], mybir.dt.float8e4),
    ]:
        ext[name] = nc.dram_tensor(name, shape, dt, kind="ExternalInput").ap()
    y_ext = nc.dram_tensor("y", [128, 4 * NBC], f32, kind="ExternalOutput").ap()
    table0 = nc.dram_tensor("table0", [NSH, D], bf16).ap()
    table1 = [nc.dram_tensor(f"table1{p}", [NSH, DH], bf16).ap() for p in range(2)]
    aggN = [[nc.dram_tensor(f"aggN{L}{p}", [NP, DH], bf16).ap() for p in range(2)]
            for L in range(2)]
    aggS = [[nc.dram_tensor(f"aggS{L}{p}", [NSH, DH], bf16).ap() for p in range(2)]
            for L in range(2)]

    RG = [list(range(NCORES))]

    with tile.TileContext(nc) as tc:
        with tc.tile_pool(name="const", bufs=1) as cp, \
             tc.tile_pool(name="hs", bufs=1) as hp, \
             tc.tile_pool(name="cv", bufs=3) as vp, \
             tc.tile_pool(name="xtp", bufs=1) as xp, \
             tc.tile_pool(name="g0", bufs=NGI) as gp0, \
             tc.tile_pool(name="g1", bufs=NGI + 2) as gp1, \
             tc.tile_pool(name="st", bufs=4) as sp, \
             tc.tile_pool(name="fv", bufs=4) as fv, \
             tc.tile_pool(name="fa", bufs=12) as fa:
            cwb = cp.tile([128, 450], bf16, tag="cwb")
            nc.sync.dma_start(cwb[:], ext["cwb"][:])
            cwf = cp.tile([128, 14], f32, tag="cwf")
            nc.sync.dma_start(cwf[:], ext["cwf"][:])
            ct = {
                "W1ab": cwb[0:8, 0:128], "W2m": cwb[:, 128:192],
                "gwd1": cwb[:, 192:320], "gwd2": cwb[:, 320:448],
                "wod2": cwb[:, 448:450],
                "b1s": cwf[:, 0:1], "b2c": cwf[0:64, 1:2],
                "gb1s": cwf[:, 2:3], "gb2s": cwf[:, 3:4],
                "dsqk": cwf[:, 4:14],
            }
            late = {}
            for name in ("eidx", "eidx1"):
                lt = cp.tile(list(ext[name].shape), ext[name].dtype, tag=name,
                             name=name)
                late[name] = lt
                ct[name] = lt
            Pt = cp.tile(list(ext["P"].shape), ext["P"].dtype, tag="P")
            ct["P"] = Pt
            ident = cp.tile([128, 128], bf16, tag="ident")
            make_identity(nc, ident[:])
            dsqd = cp.tile([128, NBC * 128], bf16, tag="dsqd")
            y_nb = cp.tile([128, 4 * NBC], f32, tag="ynb")

            hs0 = hp.tile([128, NBC * D], bf16, tag="hs0")
            hs1 = [hp.tile([128, NBC * DH], bf16, tag=f"hs1{p}",
                           name=f"hs1{p}") for p in range(2)]

            # ---- conv stage: local table0 shard = dsq * relu(conv2(relu(conv1 x)))
            with tc.tile_pool(name="c1", bufs=2, space="PSUM") as p1, \
                 tc.tile_pool(name="c2", bufs=2, space="PSUM") as p2, \
                 tc.tile_pool(name="c3", bufs=2, space="PSUM") as p3:
                xts = xp.tile([8, NBC * 512], bf16, tag="xts")
                nc.sync.dma_start(xts[:], ext["xt"][:])
                for name in ("eidx", "eidx1"):
                    nc.sync.dma_start(late[name][:], ext[name][:])
                nc.sync.dma_start(Pt[:], ext["P"][:])
                for lb in range(NBC):
                    ph1 = p1.tile([128, 512], f32, tag="ph1", space="PSUM")
                    nc.tensor.matmul(ph1[:], lhsT=ct["W1ab"][:],
                                     rhs=xts[:, lb * 512:(lb + 1) * 512],
                                     start=True, stop=True)
                    h1 = vp.tile([128, 512], bf16, tag="h1")
                    nc.vector.tensor_scalar(h1[:, 0:256], ph1[:, 0:256],
                                            ct["b1s"][:, 0:1], 0.0,
                                            mybir.AluOpType.add,
                                            mybir.AluOpType.max)
                    nc.scalar.activation(h1[:, 256:512], ph1[:, 256:512],
                                         mybir.ActivationFunctionType.Relu,
                                         bias=ct["b1s"][:, 0:1])
                    ph2 = p2.tile([64, 512], f32, tag="ph2", space="PSUM")
                    nc.tensor.matmul(ph2[:], lhsT=ct["W2m"][:], rhs=h1[:],
                                     start=True, stop=True)
                    h2 = vp.tile([64, 512], bf16, tag="h2")
                    nc.scalar.activation(h2[:, 0:256], ph2[:, 0:256],
                                         mybir.ActivationFunctionType.Relu,
                                         bias=ct["b2c"][:, 0:1])
                    nc.vector.tensor_scalar(h2[:, 256:512], ph2[:, 256:512],
                                            ct["b2c"][:, 0:1], 0.0,
                                            mybir.AluOpType.add,
                                            mybir.AluOpType.max)
                    ptp = p3.tile([128, 256], bf16, tag="ptp", space="PSUM")
                    for s in range(4):
                        nc.tensor.transpose(ptp[:, 64 * s:64 * (s + 1)],
                                            h2[:, s * 128:(s + 1) * 128],
                                            ident[0:64, 0:64])
                    if lb % 2 == 0:
                        nc.vector.tensor_scalar_mul(
                            hs0[:, lb * D:(lb + 1) * D], ptp[:],
                            ct["dsqk"][:, lb:lb + 1])
                    else:
                        nc.scalar.activation(
                            hs0[:, lb * D:(lb + 1) * D], ptp[:],
                            mybir.ActivationFunctionType.Copy,
                            scale=ct["dsqk"][:, lb:lb + 1])
                nc.sync.dma_start(
                    table0.rearrange("(lb p) f -> p lb f", p=128), hs0[:])
                for lb in range(NBC):
                    nc.vector.tensor_scalar_mul(
                        dsqd[:, lb * 128:(lb + 1) * 128], ident[:],
                        ct["dsqk"][:, lb:lb + 1])

            # ---- main pipeline PSUM pools (alive through both layers)
            with tc.tile_pool(name="sc", bufs=2, space="PSUM") as qp, \
                 tc.tile_pool(name="ps", bufs=1, space="PSUM") as ftp, \
                 tc.tile_pool(name="fb", bufs=1, space="PSUM") as ftb:
                fwp = ftp

                def emit_gathers(gp, tbl, width, idxt):
                    gts = []
                    for gi in range(NGI):
                        nsl = min(SPG, TOT - gi * SPG)
                        g = gp.tile([128, SPG, width], bf16, tag="g",
                                    name=f"g{gi}")
                        nc.gpsimd.dma_gather(
                            g[:, 0:nsl, :], tbl[:],
                            idxt[:, gi * ICPG:gi * ICPG + nsl * 8],
                            nsl * 128, nsl * 128, width)
                        gts.append(g)
                    return gts

                def emit_scatter(gts, c0, c1v, aggN_ts, par):
                    """One-hot scatter matmuls over all specs. When c1v-c0 is
                    256 both pipelines' halves are produced by one matmul and
                    staged to the two aggN tensors in aggN_ts."""
                    wid = c1v - c0
                    pb = None
                    for mmi, (b, slot, pbase, plen, first, last) in \
                            enumerate(specs):
                        g = gts[slot // SPG]
                        w = b % 5
                        if w == 0 and first:
                            pb = qp.tile([128, 5, wid], f32, tag="pb",
                                         space="PSUM", name="pb")
                        nc.tensor.matmul(
                            pb[:, w, :],
                            lhsT=ct["P"][:, mmi * 128:(mmi + 1) * 128],
                            rhs=g[:, slot % SPG, c0:c1v],
                            start=first, stop=last)
                        if last and w == 4:
                            grp = b // 5
                            for hi, aggN_t in enumerate(aggN_ts):
                                stg = sp.tile([128, 5, DH], bf16, tag="stg")
                                if (grp + par + hi) % 2 == 0:
                                    nc.vector.tensor_copy(
                                        stg[:],
                                        pb[:, :, hi * DH:hi * DH + DH])
                                else:
                                    nc.scalar.activation(
                                        stg[:],
                                        pb[:, :, hi * DH:hi * DH + DH],
                                        mybir.ActivationFunctionType.Copy)
                                nc.sync.dma_start(
                                    aggN_t[grp * 640:(grp + 1) * 640, :]
                                    .rearrange("(p q) f -> p q f", p=128),
                                    stg[:])

                def emit_finish(L, p):
                    """Self-loop + dsq + W + bias + relu for pipeline p."""
                    gwd = ct["gwd1"] if L == 0 else ct["gwd2"]
                    gbs = ct["gb1s"] if L == 0 else ct["gb2s"]
                    hs_cur = hs0 if L == 0 else hs1[p]
                    agg = aggS[L][p]
                    asbs = []
                    for pi in range(NBC // 2):
                        asb = fa.tile([128, 2, DH], bf16, tag="asb",
                                      name=f"asb{L}{p}{pi}")
                        b0 = 2 * pi
                        g0_, q0 = divmod(b0, 5)
                        g1_, q1 = divmod(b0 + 1, 5)
                        if g0_ == g1_:
                            nc.sync.dma_start(
                                asb[:],
                                agg[g0_ * 640:(g0_ + 1) * 640, :]
                                .rearrange("(p q) f -> p q f", p=128)
                                [:, q0:q0 + 2, :])
                        else:
                            nc.sync.dma_start(
                                asb[:, 0:1, :],
                                agg[g0_ * 640:(g0_ + 1) * 640, :]
                                .rearrange("(p q) f -> p q f", p=128)
                                [:, q0:q0 + 1, :])
                            nc.sync.dma_start(
                                asb[:, 1:2, :],
                                agg[g1_ * 640:(g1_ + 1) * 640, :]
                                .rearrange("(p q) f -> p q f", p=128)
                                [:, q1:q1 + 1, :])
                        asbs.append(asb)
                    for pi in range(NBC // 2):
                        asb = asbs[pi]
                        tp2 = ftp.tile([128, 2 * DH], f32, tag="ps",
                                       space="PSUM", name="tp2")
                        for c in range(2):
                            b = 2 * pi + c
                            if L == 0:
                                hsl = hs_cur[:, b * D + p * DH:
                                             b * D + (p + 1) * DH]
                            else:
                                hsl = hs_cur[:, b * DH:(b + 1) * DH]
                            nc.tensor.matmul(
                                tp2[:, c * DH:(c + 1) * DH],
                                lhsT=asb[:, c, :],
                                rhs=dsqd[:, b * 128:(b + 1) * 128],
                                start=True, stop=False)
                            nc.tensor.matmul(
                                tp2[:, c * DH:(c + 1) * DH],
                                lhsT=hsl,
                                rhs=dsqd[:, b * 128:(b + 1) * 128],
                                start=False, stop=True)
                        tps = fv.tile([128, 2 * DH], bf16, tag="tps")
                        if pi % 2 == 0:
                            nc.vector.tensor_copy(tps[:], tp2[:])
                        else:
                            nc.scalar.activation(
                                tps[:], tp2[:],
                                mybir.ActivationFunctionType.Copy)
                        wp2 = fwp.tile([128, 2 * DH], f32, tag="ps",
                                       space="PSUM", name="wp2")
                        nc.tensor.matmul(wp2[:], lhsT=gwd[:], rhs=tps[:],
                                         start=True, stop=True)
                        h42 = fv.tile([128, 2 * DH], bf16, tag="h42")
                        nc.scalar.activation(h42[:], wp2[:],
                                             mybir.ActivationFunctionType.Relu,
                                             bias=gbs[:, 0:1])
                        if L == 0:
                            tb2 = ftb.tile([128, 2, 128], bf16, tag="fb",
                                           space="PSUM", name="tb2")
                            for c in range(2):
                                nc.tensor.transpose(
                                    tb2[:, c, :],
                                    h42[:, c * 128:(c + 1) * 128], ident[:])
                            hsn = hs1[p]
                            for c in range(2):
                                b = 2 * pi + c
                                if c == 0:
                                    nc.vector.tensor_scalar_mul(
                                        hsn[:, b * DH:(b + 1) * DH],
                                        tb2[:, c, :],
                                        ct["dsqk"][:, b:b + 1])
                                else:
                                    nc.scalar.activation(
                                        hsn[:, b * DH:(b + 1) * DH],
                                        tb2[:, c, :],
                                        mybir.ActivationFunctionType.Copy,
                                        scale=ct["dsqk"][:, b:b + 1])
                            # permuted rows: p*2 + lb within the bp
                            nc.sync.dma_start(
                                table1[p][pi * 256:(pi + 1) * 256, :]
                                .rearrange("(p lb) f -> p (lb f)", p=128),
                                hsn[:, pi * 2 * DH:(pi + 1) * 2 * DH])
                        else:
                            yp2 = ftb.tile([128, 4], f32, tag="fb",
                                           space="PSUM", name="yp2")
                            for c in range(2):
                                nc.tensor.matmul(
                                    yp2[:, c * 2:(c + 1) * 2],
                                    lhsT=h42[:, c * 128:(c + 1) * 128],
                                    rhs=ct["wod2"][:],
                                    start=True, stop=True)
                            for c in range(2):
                                b = 2 * pi + c
                                nc.vector.tensor_scalar_add(
                                    y_nb[:, 4 * b + 2 * p:4 * b + 2 * p + 2],
                                    yp2[:, c * 2:(c + 1) * 2], bo_f)

                # ---- pipeline schedule
                gts0 = emit_gathers(gp0, table0, D, ct["eidx"])
                emit_scatter(gts0, 0, D, [aggN[0][0], aggN[0][1]], 0)
                nc.gpsimd.collective_compute(
                    "ReduceScatter", mybir.AluOpType.add, replica_groups=RG,
                    ins=[aggN[0][0][:]], outs=[aggS[0][0][:]])
                nc.gpsimd.collective_compute(
                    "ReduceScatter", mybir.AluOpType.add, replica_groups=RG,
                    ins=[aggN[0][1][:]], outs=[aggS[0][1][:]])
                emit_finish(0, 0)
                gts1a = emit_gathers(gp1, table1[0], DH, ct["eidx1"])
                emit_scatter(gts1a, 0, DH, [aggN[1][0]], 0)
                emit_finish(0, 1)
                nc.gpsimd.collective_compute(
                    "ReduceScatter", mybir.AluOpType.add, replica_groups=RG,
                    ins=[aggN[1][0][:]], outs=[aggS[1][0][:]])
                gts1b = emit_gathers(gp1, table1[1], DH, ct["eidx1"])
                emit_scatter(gts1b, 0, DH, [aggN[1][1]], 1)
                nc.gpsimd.collective_compute(
                    "ReduceScatter", mybir.AluOpType.add, replica_groups=RG,
                    ins=[aggN[1][1][:]], outs=[aggS[1][1][:]])
                emit_finish(1, 0)
                emit_finish(1, 1)

            nc.sync.dma_start(y_ext[:], y_nb[:])
    nc.compile()
    return nc


def _run(inputs):
    from concourse.bass_utils import run_bass_kernel_spmd

    in_maps, specs, TOT, bo_f, rown = _host_prep(
        inputs["x"], inputs["edge_index"], inputs["w1"], inputs["b1"],
        inputs["w2"], inputs["b2"], inputs["gw1"], inputs["gb1"],
        inputs["gw2"], inputs["gb2"], inputs["wo"], inputs["bo"])

    key = (hash(specs), TOT)
    if key not in _cache:
        _cache[key] = _build(specs, TOT, bo_f)
    nc = _cache[key]

    res = run_bass_kernel_spmd(nc, in_maps, list(range(8)))
    Yall = np.zeros((NP, B), dtype=np.float32)
    for k in range(NCORES):
        y_nb = res.results[k]["y"]          # [128, 4*NBC]
        for lb in range(NBC):
            lo = k * NSH + lb * 128
            for s in range(B):
                Yall[lo:lo + 128, s] = y_nb[:, lb * 4 + s]
    return Yall[rown, :].T.copy()


def kernel(**inputs):
    return _run(inputs)


# revision 37
# speedup vs baseline: 1.1014x; 1.1014x over previous
"""GraphWaveNet kernel for Trainium2 (Bass/Tile), 8 NeuronCores.

Design: edge sharding by SOURCE block, dense slot packing, and TWO
independent slice-pair pipelines (A = batch slices 0,1; B = 2,3) so the
four per-pipeline ReduceScatters interleave with each other's compute.

- Only t=11 survives the final 1x1 conv and the GCN doesn't mix time, so
  the conv stack is evaluated at t in {10,11} only and the GCN runs on
  B=4 slices packed as 256 columns (4 slices x 64 feats); pipeline A
  owns cols 0:128, B owns 128:256. The slices never mix, so A and B are
  fully independent after the conv -- their collectives pipeline:
    g0 A0 B0 RS0A [RS0B | fin0A g1a A1] RS1A [fin0B g1b B1] RS1B [fin1A] fin1B
- GCN identity: with Hs = dsq*h, agg_n = dsq_n * (sum_{e->n} Hs[src_e]
  + Hs[n]), then @W + b + relu.
- Sharding: core k owns node rows [1280k, 1280(k+1)). Edges live on the
  core owning their SRC, so gathers (dma_gather) read only the local
  table. Dense slot packing: block b's edges sit at global positions
  [S[b], S[b]+maxcnt[b]) where maxcnt = max over cores (SPMD-uniform
  schedule); specs are the 128-boundary pieces, each one one-hot P
  matmul into the block's PSUM accumulator (5-block group tiles, copy +
  DMA per group) forming a bf16 partial aggregate over all 10240 rows.
  One ReduceScatter per (layer, pipeline) returns each core its rows.
- PSUM chains: specs are emitted in position (= block-major) order, so
  a chain closes before the next block's opens -- never two open chains
  in one bank.
- Layer 0 gathers the full 256-col table once; both pipelines' scatter
  matmuls slice the same gathered tiles. Layer 1 tables are per-pipeline
  (128 cols), gathered separately with the same slot positions (eidx1).
- HBM layouts are chosen for >=512B contiguous DMA runs: aggN group
  regions are (p-major, q-minor) so each partition writes 1280B; table1
  rows are permuted (per 256-node bp: row = p*2 + lb) so finish writes
  512B runs; eidx1 bakes the permutation into the gather indices.
- dma_gather ucode contract (queue 0): flat index j of an instruction
  lands at out[j%128, j//128] and is read from idx tile position
  [16 + j%16, j//16] (int16).
"""

import sys

sys.path.insert(0, "/opt/trn_rl_repo")

import numpy as np
import ml_dtypes

B, T, N, FIN, H, E = 4, 12, 10000, 2, 64, 80000
NCORES = 8
NB80 = 80                 # dst blocks of 128 nodes
NP = NB80 * 128           # padded node count (10240)
NSH = NP // NCORES        # node rows per core (1280)
NBC = NB80 // NCORES      # node blocks per core (10)
D = 4 * H                 # 256 = 4 slices x 64 feats
DH = D // 2               # 128 = one pipeline's cols
SPG = 8                   # slots (of 128 edges) per dma_gather (1024 idxs)
NIG = SPG * 128           # indices per full gather instruction
ICPG = NIG // 16          # idx tile columns per full gather instr (64)

_cache = {}


def _balance(src, dst):
    """Node -> row permutation so every (core, dst-block) edge count <= 128.
    Phase 1: swap nodes between cores until pairwise core->core edge counts
    fit 10 blocks x 128. Phase 2: per-core greedy min-max packing into 10
    blocks with a repair pass. Heuristic: any residual overflow is handled
    by the dense spec packing (extra matmul pieces), not a correctness issue.
    """
    NPC = N // NCORES + (1 if N % NCORES else 0)   # 1250
    kc = np.minimum(np.arange(N) // NPC, NCORES - 1)
    out_deg = np.bincount(src, minlength=N)

    def mkT(kc):
        T = np.zeros((NCORES, NCORES), np.int64)
        np.add.at(T, (kc[src], kc[dst]), 1)
        return T

    T = mkT(kc)
    for _ in range(400):
        k, c = np.unravel_index(np.argmax(T), T.shape)
        if T[k, c] <= 1270:
            break
        ink = np.bincount(dst[kc[src] == k], minlength=N)
        nodes_c = np.flatnonzero(kc == c)
        n = nodes_c[np.argmax(ink[nodes_c] - 0.1 * out_deg[nodes_c])]
        c2 = np.argmin(T[k] + np.where(np.arange(NCORES) == c, 10 ** 9, 0))
        nodes_c2 = np.flatnonzero(kc == c2)
        n2 = nodes_c2[np.argmin(ink[nodes_c2] + 0.1 * out_deg[nodes_c2])]
        kc[n], kc[n2] = c2, c
        T = mkT(kc)

    dvec = np.zeros((N, NCORES), np.int64)
    np.add.at(dvec, (dst, kc[src]), 1)
    row_of_node = np.full(N, -1, np.int64)
    for c in range(NCORES):
        nodes = np.flatnonzero(kc == c)
        vv = dvec[nodes]
        order = np.argsort(-vv.sum(1), kind="stable")
        load = np.zeros((NBC, NCORES), np.int64)
        nn = np.zeros(NBC, np.int64)
        assign = np.zeros(len(nodes), np.int64)
        for i in order:
            v = vv[i]
            cand = np.flatnonzero(nn < 128)
            newmax = (load[cand] + v).max(axis=1)
            ok = newmax <= 128
            if ok.any():
                c2_ = cand[ok]
                b = c2_[np.argmin((load[c2_] + v).max(axis=1) * 1000
                                  + nn[c2_])]
            else:
                b = cand[np.argmin(newmax)]
            assign[i] = b
            load[b] += v
            nn[b] += 1
        for _ in range(300):
            viol = np.argwhere(load > 128)
            if len(viol) == 0:
                break
            b, k = viol[np.argmax(load[viol[:, 0], viol[:, 1]])]
            members = np.flatnonzero(assign == b)
            cand_n = members[vv[members, k] > 0]
            cand_n = cand_n[np.argsort(-vv[cand_n, k])]
            done = False
            for i in cand_n[:20]:
                v = vv[i]
                tgt = np.flatnonzero((nn < 128)
                                     & ((load + v) <= 128).all(axis=1))
                tgt = tgt[tgt != b]
                if len(tgt):
                    t = tgt[np.argmin((load[tgt] + v).max(axis=1))]
                    assign[i] = t
                    load[b] -= v
                    load[t] += v
                    nn[b] -= 1
                    nn[t] += 1
                    done = True
                    break
            if not done:
                break
        for b in range(NBC):
            sel = nodes[assign == b]
            base = c * NSH + b * 128
            row_of_node[sel] = base + np.arange(len(sel))
    return row_of_node


def _host_prep(x, edge_index, w1, b1, w2, b2, gw1, gb1, gw2, gb2, wo, bo):
    x = np.asarray(x, np.float32)
    src0 = np.asarray(edge_index[0]).astype(np.int64)
    dst0 = np.asarray(edge_index[1]).astype(np.int64)
    rown = _balance(src0, dst0)
    src, dst = rown[src0], rown[dst0]

    deg = np.bincount(dst0, minlength=N).astype(np.float64) + 1.0
    dsq = (deg ** -0.5).astype(np.float32)
    dsq_pad = np.ones(NP, dtype=np.float32)
    dsq_pad[rown] = dsq

    # ---- per-core edge partition by src owner, dst-sorted
    owner = src // NSH
    es_k, ed_k, cnt = [], [], np.zeros((NCORES, NB80), np.int64)
    for k in range(NCORES):
        m = owner == k
        es, ed = src[m], dst[m]
        o = np.argsort(ed, kind="stable")
        es_k.append(es[o])
        ed_k.append(ed[o])
        cnt[k] = np.bincount(ed[o] // 128, minlength=NB80)

    # dense schedule: block b's edges at positions [S[b], S[b]+mc[b])
    mc = np.maximum(1, cnt.max(axis=0))          # SPMD-uniform per block
    S = np.zeros(NB80 + 1, np.int64)
    S[1:] = np.cumsum(mc)
    TOTE = int(S[NB80])
    TOT = (TOTE + 127) // 128                    # slots
    # specs: 128-boundary pieces of each block segment, in position order
    specs = []                                   # (block, slot, pbase, plen, first, last)
    for b in range(NB80):
        a, e = int(S[b]), int(S[b] + mc[b])
        p = a
        while p < e:
            q = min(e, (p // 128 + 1) * 128)
            specs.append((b, p // 128, p % 128, q - p, p == a, q == e))
            p = q
    NMM = len(specs)

    # eidx / eidx1 / P (per-core data; schedule above is uniform)
    NGI = (TOT + SPG - 1) // SPG
    COLS = NGI * ICPG
    eidx_all = np.zeros((NCORES, 128, COLS), np.int16)
    eidx1_all = np.zeros((NCORES, 128, COLS), np.int16)
    P_all = np.zeros((NCORES, 128, NMM * 128), np.float32)
    for k in range(NCORES):
        es, ed = es_k[k], ed_k[k]
        bounds = np.searchsorted(ed, np.arange(NB80 + 1) * 128)
        for b in range(NB80):
            e0, e1 = int(bounds[b]), int(bounds[b + 1])
            ne = e1 - e0
            if ne == 0:
                continue
            jj = int(S[b]) + np.arange(ne)       # global positions
            rows = 16 + (jj % 16)
            cols = (jj // NIG) * ICPG + (jj % NIG) // 16
            loc = (es[e0:e1] - k * NSH).astype(np.int64)
            eidx_all[k, rows, cols] = loc.astype(np.int16)
            # table1 physical row: per 256-node bp, row = p*2 + lb
            lbg, p = loc // 128, loc % 128
            phys = (lbg // 2) * 256 + p * 2 + (lbg % 2)
            eidx1_all[k, rows, cols] = phys.astype(np.int16)
        for mmi, (b, slot, pbase, plen, first, last) in enumerate(specs):
            gpos = slot * 128 + pbase            # global position of spec start
            e0 = int(bounds[b]) + (gpos - int(S[b]))
            ne = min(plen, int(bounds[b + 1]) - e0)
            if ne > 0:
                P_all[k, pbase + np.arange(ne),
                      mmi * 128 + (ed[e0:e0 + ne] - b * 128)] = 1.0
    P_all = P_all.astype(ml_dtypes.float8_e4m3fn)   # one-hot: 1.0 exact in fp8

    # ---- conv input: per block 8 rows (t,c) for t in {9,10,11} + 2 zero rows,
    # cols = 4 slices x 128 nodes
    xpad = np.zeros((B, 3, FIN, NP), np.float32)
    xpad[:, :, :, rown] = x[:, 9:12, :, :].transpose(0, 1, 3, 2)  # [s, ti, c, n]
    xv = xpad.reshape(B, 6, NCORES, NBC, 128)                   # [s, row, k, blk, p]
    xt_all = np.zeros((NCORES, 8, NBC * 4 * 128), np.float32)
    xt_all[:, :6] = xv.transpose(2, 1, 3, 0, 4).reshape(NCORES, 6, NBC * 4 * 128)
    xt_all = xt_all.astype(ml_dtypes.bfloat16)

    dsqk_all = dsq_pad.reshape(NCORES, NBC, 128).transpose(0, 2, 1).copy()

    # ---- weights
    W1m = np.zeros((6, 64), np.float32)
    for kk in range(3):
        for c in range(FIN):
            W1m[2 * kk + c, :] = w1[:, c, 0, kk]
    W1ab = np.zeros((8, 128), np.float32)
    W1ab[0:6, 0:64] = W1m          # A: t10 (taps t9,t10,t11)
    W1ab[2:8, 64:128] = W1m        # B: t11 (taps t10,t11,t12=pad)
    W1ab = W1ab.astype(ml_dtypes.bfloat16)

    W2m = np.zeros((128, 64), np.float32)
    W2m[:64, :] = w2[:, :, 0, 0].T
    W2m[64:, :] = w2[:, :, 0, 1].T
    W2m = W2m.astype(ml_dtypes.bfloat16)

    b1s = np.concatenate([b1, b1]).reshape(128, 1).astype(np.float32)
    b2c = np.asarray(b2, np.float32).reshape(64, 1)
    gb1s = np.concatenate([gb1, gb1]).reshape(128, 1).astype(np.float32)
    gb2s = np.concatenate([gb2, gb2]).reshape(128, 1).astype(np.float32)
    gwd1 = np.zeros((128, 128), np.float32)
    gwd1[0:64, 0:64] = gw1
    gwd1[64:128, 64:128] = gw1
    gwd1 = gwd1.astype(ml_dtypes.bfloat16)
    gwd2 = np.zeros((128, 128), np.float32)
    gwd2[0:64, 0:64] = gw2
    gwd2[64:128, 64:128] = gw2
    gwd2 = gwd2.astype(ml_dtypes.bfloat16)
    wov = np.asarray(wo, np.float32)[0, :, 0, 0]
    wod2 = np.zeros((128, 2), np.float32)
    wod2[0:64, 0] = wov
    wod2[64:128, 1] = wov
    wod2 = wod2.astype(ml_dtypes.bfloat16)

    # pack all small weights into two tensors (one DMA each)
    cwb = np.zeros((128, 450), ml_dtypes.bfloat16)
    cwb[0:8, 0:128] = W1ab
    cwb[:, 128:192] = W2m
    cwb[:, 192:320] = gwd1
    cwb[:, 320:448] = gwd2
    cwb[:, 448:450] = wod2
    cwf = np.zeros((128, 14), np.float32)
    cwf[:, 0:1] = b1s
    cwf[0:64, 1:2] = b2c
    cwf[:, 2:3] = gb1s
    cwf[:, 3:4] = gb2s

    shared = {"cwb": cwb, "cwf": cwf}
    in_maps = []
    for k in range(NCORES):
        m = dict(shared)
        cf = m["cwf"].copy()
        cf[:, 4:14] = dsqk_all[k]
        m["cwf"] = cf
        m["xt"] = xt_all[k]
        m["eidx"] = eidx_all[k]
        m["eidx1"] = eidx1_all[k]
        m["P"] = P_all[k]
        in_maps.append(m)
    return (in_maps, tuple(specs), TOT,
            float(np.asarray(bo).reshape(-1)[0]), rown)


def _build(specs, TOT, bo_f):
    from concourse import bass, bacc, tile
    from concourse.masks import make_identity
    import mybir

    f32, bf16, i16 = mybir.dt.float32, mybir.dt.bfloat16, mybir.dt.int16
    NMM = len(specs)
    NGI = (TOT + SPG - 1) // SPG
    COLS = NGI * ICPG

    nc = bacc.Bacc("TRN2", target_bir_lowering=False, debug=False, num_devices=8,
                   dynamic_dma_scratch_size=65536)

    ext = {}
    for name, shape, dt in [
        ("xt", [8, NBC * 512], bf16), ("cwb", [128, 450], bf16),
        ("cwf", [128, 14], f32),
        ("eidx", [128, COLS], i16), ("eidx1", [128, COLS], i16),
        ("P", [128, NMM * 128], mybir.dt.float8e4),
    ]:
        ext[name] = nc.dram_tensor(name, shape, dt, kind="ExternalInput").ap()
    y_ext = nc.dram_tensor("y", [128, 4 * NBC], f32, kind="ExternalOutput").ap()
    table0 = nc.dram_tensor("table0", [NSH, D], bf16).ap()
    table1 = [nc.dram_tensor(f"table1{p}", [NSH, DH], bf16).ap() for p in range(2)]
    aggN = [[nc.dram_tensor(f"aggN{L}{p}", [NP, DH], bf16).ap() for p in range(2)]
            for L in range(2)]
    aggS = [[nc.dram_tensor(f"aggS{L}{p}", [NSH, DH], bf16).ap() for p in range(2)]
            for L in range(2)]

    RG = [list(range(NCORES))]

    with tile.TileContext(nc) as tc:
        with tc.tile_pool(name="const", bufs=1) as cp, \
             tc.tile_pool(name="hs", bufs=1) as hp, \
             tc.tile_pool(name="cv", bufs=3) as vp, \
             tc.tile_pool(name="xtp", bufs=1) as xp, \
             tc.tile_pool(name="g0", bufs=NGI) as gp0, \
             tc.tile_pool(name="g1", bufs=NGI + 2) as gp1, \
             tc.tile_pool(name="st", bufs=4) as sp, \
             tc.tile_pool(name="fv", bufs=4) as fv, \
             tc.tile_pool(name="fa", bufs=12) as fa:
            cwb = cp.tile([128, 450], bf16, tag="cwb")
            nc.sync.dma_start(cwb[:], ext["cwb"][:])
            cwf = cp.tile([128, 14], f32, tag="cwf")
            nc.sync.dma_start(cwf[:], ext["cwf"][:])
            ct = {
                "W1ab": cwb[0:8, 0:128], "W2m": cwb[:, 128:192],
                "gwd1": cwb[:, 192:320], "gwd2": cwb[:, 320:448],
                "wod2": cwb[:, 448:450],
                "b1s": cwf[:, 0:1], "b2c": cwf[0:64, 1:2],
                "gb1s": cwf[:, 2:3], "gb2s": cwf[:, 3:4],
                "dsqk": cwf[:, 4:14],
            }
            late = {}
            for name in ("eidx", "eidx1"):
                lt = cp.tile(list(ext[name].shape), ext[name].dtype, tag=name,
                             name=name)
                late[name] = lt
                ct[name] = lt
            Pt = cp.tile(list(ext["P"].shape), ext["P"].dtype, tag="P")
            ct["P"] = Pt
            ident = cp.tile([128, 128], bf16, tag="ident")
            make_identity(nc, ident[:])
            dsqd = cp.tile([128, NBC * 128], bf16, tag="dsqd")
            y_nb = cp.tile([128, 4 * NBC], f32, tag="ynb")

            hs0 = hp.tile([128, NBC * D], bf16, tag="hs0")
            hs1 = [hp.tile([128, NBC * DH], bf16, tag=f"hs1{p}",
                           name=f"hs1{p}") for p in range(2)]

            # ---- conv stage: local table0 shard = dsq * relu(conv2(relu(conv1 x)))
            with tc.tile_pool(name="c1", bufs=2, space="PSUM") as p1, \
                 tc.tile_pool(name="c2", bufs=2, space="PSUM") as p2, \
                 tc.tile_pool(name="c3", bufs=2, space="PSUM") as p3:
                xts = xp.tile([8, NBC * 512], bf16, tag="xts")
                nc.sync.dma_start(xts[:], ext["xt"][:])
                for name in ("eidx", "eidx1"):
                    nc.sync.dma_start(late[name][:], ext[name][:])
                nc.sync.dma_start(Pt[:], ext["P"][:])
                for lb in range(NBC):
                    ph1 = p1.tile([128, 512], f32, tag="ph1", space="PSUM")
                    nc.tensor.matmul(ph1[:], lhsT=ct["W1ab"][:],
                                     rhs=xts[:, lb * 512:(lb + 1) * 512],
                                     start=True, stop=True)
                    h1 = vp.tile([128, 512], bf16, tag="h1")
                    nc.vector.tensor_scalar(h1[:, 0:256], ph1[:, 0:256],
                                            ct["b1s"][:, 0:1], 0.0,
                                            mybir.AluOpType.add,
                                            mybir.AluOpType.max)
                    nc.scalar.activation(h1[:, 256:512], ph1[:, 256:512],
                                         mybir.ActivationFunctionType.Relu,
                                         bias=ct["b1s"][:, 0:1])
                    ph2 = p2.tile([64, 512], f32, tag="ph2", space="PSUM")
                    nc.tensor.matmul(ph2[:], lhsT=ct["W2m"][:], rhs=h1[:],
                                     start=True, stop=True)
                    h2 = vp.tile([64, 512], bf16, tag="h2")
                    nc.scalar.activation(h2[:, 0:256], ph2[:, 0:256],
                                         mybir.ActivationFunctionType.Relu,
                                         bias=ct["b2c"][:, 0:1])
                    nc.vector.tensor_scalar(h2[:, 256:512], ph2[:, 256:512],
                                            ct["b2c"][:, 0:1], 0.0,
                                            mybir.AluOpType.add,
                                            mybir.AluOpType.max)
                    ptp = p3.tile([128, 256], bf16, tag="ptp", space="PSUM")
                    for s in range(4):
                        nc.tensor.transpose(ptp[:, 64 * s:64 * (s + 1)],
                                            h2[:, s * 128:(s + 1) * 128],
                                            ident[0:64, 0:64])
                    if lb % 2 == 0:
                        nc.vector.tensor_scalar_mul(
                            hs0[:, lb * D:(lb + 1) * D], ptp[:],
                            ct["dsqk"][:, lb:lb + 1])
                    else:
                        nc.scalar.activation(
                            hs0[:, lb * D:(lb + 1) * D], ptp[:],
                            mybir.ActivationFunctionType.Copy,
                            scale=ct["dsqk"][:, lb:lb + 1])
                nc.sync.dma_start(
                    table0.rearrange("(lb p) f -> p lb f", p=128), hs0[:])
                for lb in range(NBC):
                    nc.vector.tensor_scalar_mul(
                        dsqd[:, lb * 128:(lb + 1) * 128], ident[:],
                        ct["dsqk"][:, lb:lb + 1])

            # ---- main pipeline PSUM pools (alive through both layers)
            with tc.tile_pool(name="sc", bufs=2, space="PSUM") as qp, \
                 tc.tile_pool(name="ps", bufs=3, space="PSUM") as ftp, \
                 tc.tile_pool(name="fb", bufs=1, space="PSUM") as ftb:
                fwp = ftp

                def emit_gathers(gp, tbl, width, idxt):
                    gts = []
                    for gi in range(NGI):
                        nsl = min(SPG, TOT - gi * SPG)
                        g = gp.tile([128, SPG, width], bf16, tag="g",
                                    name=f"g{gi}")
                        nc.gpsimd.dma_gather(
                            g[:, 0:nsl, :], tbl[:],
                            idxt[:, gi * ICPG:gi * ICPG + nsl * 8],
                            nsl * 128, nsl * 128, width)
                        gts.append(g)
                    return gts

                def emit_scatter(gts, c0, c1v, aggN_ts, par):
                    """One-hot scatter matmuls over all specs. When c1v-c0 is
                    256 both pipelines' halves are produced by one matmul and
                    staged to the two aggN tensors in aggN_ts."""
                    wid = c1v - c0
                    pb = None
                    for mmi, (b, slot, pbase, plen, first, last) in \
                            enumerate(specs):
                        g = gts[slot // SPG]
                        w = b % 5
                        if w == 0 and first:
                            pb = qp.tile([128, 5, wid], f32, tag="pb",
                                         space="PSUM", name="pb")
                        nc.tensor.matmul(
                            pb[:, w, :],
                            lhsT=ct["P"][:, mmi * 128:(mmi + 1) * 128],
                            rhs=g[:, slot % SPG, c0:c1v],
                            start=first, stop=last)
                        if last and w == 4:
                            grp = b // 5
                            for hi, aggN_t in enumerate(aggN_ts):
                                stg = sp.tile([128, 5, DH], bf16, tag="stg")
                                if (grp + par + hi) % 2 == 0:
                                    nc.vector.tensor_copy(
                                        stg[:],
                                        pb[:, :, hi * DH:hi * DH + DH])
                                else:
                                    nc.scalar.activation(
                                        stg[:],
                                        pb[:, :, hi * DH:hi * DH + DH],
                                        mybir.ActivationFunctionType.Copy)
                                nc.sync.dma_start(
                                    aggN_t[grp * 640:(grp + 1) * 640, :]
                                    .rearrange("(p q) f -> p q f", p=128),
                                    stg[:])

                def emit_finish(L, p):
                    """Self-loop + dsq + W + bias + relu for pipeline p."""
                    gwd = ct["gwd1"] if L == 0 else ct["gwd2"]
                    gbs = ct["gb1s"] if L == 0 else ct["gb2s"]
                    hs_cur = hs0 if L == 0 else hs1[p]
                    agg = aggS[L][p]
                    asbs = []
                    for pi in range(NBC // 2):
                        asb = fa.tile([128, 2, DH], bf16, tag="asb",
                                      name=f"asb{L}{p}{pi}")
                        b0 = 2 * pi
                        g0_, q0 = divmod(b0, 5)
                        g1_, q1 = divmod(b0 + 1, 5)
                        if g0_ == g1_:
                            nc.sync.dma_start(
                                asb[:],
                                agg[g0_ * 640:(g0_ + 1) * 640, :]
                                .rearrange("(p q) f -> p q f", p=128)
                                [:, q0:q0 + 2, :])
                        else:
                            nc.sync.dma_start(
                                asb[:, 0:1, :],
                                agg[g0_ * 640:(g0_ + 1) * 640, :]
                                .rearrange("(p q) f -> p q f", p=128)
                                [:, q0:q0 + 1, :])
                            nc.sync.dma_start(
                                asb[:, 1:2, :],
                                agg[g1_ * 640:(g1_ + 1) * 640, :]
                                .rearrange("(p q) f -> p q f", p=128)
                                [:, q1:q1 + 1, :])
                        asbs.append(asb)
                    for pi in range(NBC // 2):
                        asb = asbs[pi]
                        tp2 = ftp.tile([128, 2 * DH], f32, tag="ps",
                                       space="PSUM", name="tp2")
                        for c in range(2):
                            b = 2 * pi + c
                            if L == 0:
                                hsl = hs_cur[:, b * D + p * DH:
                                             b * D + (p + 1) * DH]
                            else:
                                hsl = hs_cur[:, b * DH:(b + 1) * DH]
                            nc.tensor.matmul(
                                tp2[:, c * DH:(c + 1) * DH],
                                lhsT=asb[:, c, :],
                                rhs=dsqd[:, b * 128:(b + 1) * 128],
                                start=True, stop=False)
                            nc.tensor.matmul(
                                tp2[:, c * DH:(c + 1) * DH],
                                lhsT=hsl,
                                rhs=dsqd[:, b * 128:(b + 1) * 128],
                                start=False, stop=True)
                        tps = fv.tile([128, 2 * DH], bf16, tag="tps")
                        if pi % 2 == 0:
                            nc.vector.tensor_copy(tps[:], tp2[:])
                        else:
                            nc.scalar.activation(
                                tps[:], tp2[:],
                                mybir.ActivationFunctionType.Copy)
                        wp2 = fwp.tile([128, 2 * DH], f32, tag="ps",
                                       space="PSUM", name="wp2")
                        nc.tensor.matmul(wp2[:], lhsT=gwd[:], rhs=tps[:],
                                         start=True, stop=True)
                        h42 = fv.tile([128, 2 * DH], bf16, tag="h42")
                        nc.scalar.activation(h42[:], wp2[:],
                                             mybir.ActivationFunctionType.Relu,
                                             bias=gbs[:, 0:1])
                        if L == 0:
                            tb2 = ftb.tile([128, 2, 128], bf16, tag="fb",
                                           space="PSUM", name="tb2")
                            for c in range(2):
                                nc.tensor.transpose(
                                    tb2[:, c, :],
                                    h42[:, c * 128:(c + 1) * 128], ident[:])
                            hsn = hs1[p]
                            for c in range(2):
                                b = 2 * pi + c
                                if c == 0:
                                    nc.vector.tensor_scalar_mul(
                                        hsn[:, b * DH:(b + 1) * DH],
                                        tb2[:, c, :],
                                        ct["dsqk"][:, b:b + 1])
                                else:
                                    nc.scalar.activation(
                                        hsn[:, b * DH:(b + 1) * DH],
                                        tb2[:, c, :],
                                        mybir.ActivationFunctionType.Copy,
                                        scale=ct["dsqk"][:, b:b + 1])
                            # permuted rows: p*2 + lb within the bp
                            nc.sync.dma_start(
                                table1[p][pi * 256:(pi + 1) * 256, :]
                                .rearrange("(p lb) f -> p (lb f)", p=128),
                                hsn[:, pi * 2 * DH:(pi + 1) * 2 * DH])
                        else:
                            yp2 = ftb.tile([128, 4], f32, tag="fb",
                                           space="PSUM", name="yp2")
                            for c in range(2):
                                nc.tensor.matmul(
                                    yp2[:, c * 2:(c + 1) * 2],
                                    lhsT=h42[:, c * 128:(c + 1) * 128],
                                    rhs=ct["wod2"][:],
                                    start=True, stop=True)
                            for c in range(2):
                                b = 2 * pi + c
                                nc.vector.tensor_scalar_add(
                                    y_nb[:, 4 * b + 2 * p:4 * b + 2 * p + 2],
                                    yp2[:, c * 2:(c + 1) * 2], bo_f)

                # ---- pipeline schedule
                gts0 = emit_gathers(gp0, table0, D, ct["eidx"])
                emit_scatter(gts0, 0, DH, [aggN[0][0]], 0)
                emit_scatter(gts0, DH, D, [aggN[0][1]], 1)
                nc.gpsimd.collective_compute(
                    "ReduceScatter", mybir.AluOpType.add, replica_groups=RG,
                    ins=[aggN[0][0][:]], outs=[aggS[0][0][:]])
                nc.gpsimd.collective_compute(
                    "ReduceScatter", mybir.AluOpType.add, replica_groups=RG,
                    ins=[aggN[0][1][:]], outs=[aggS[0][1][:]])
                emit_finish(0, 0)
                gts1a = emit_gathers(gp1, table1[0], DH, ct["eidx1"])
                emit_scatter(gts1a, 0, DH, [aggN[1][0]], 0)
                emit_finish(0, 1)
                nc.gpsimd.collective_compute(
                    "ReduceScatter", mybir.AluOpType.add, replica_groups=RG,
                    ins=[aggN[1][0][:]], outs=[aggS[1][0][:]])
                gts1b = emit_gathers(gp1, table1[1], DH, ct["eidx1"])
                emit_scatter(gts1b, 0, DH, [aggN[1][1]], 1)
                nc.gpsimd.collective_compute(
                    "ReduceScatter", mybir.AluOpType.add, replica_groups=RG,
                    ins=[aggN[1][1][:]], outs=[aggS[1][1][:]])
                emit_finish(1, 0)
                emit_finish(1, 1)

            nc.sync.dma_start(y_ext[:], y_nb[:])
    nc.compile()
    return nc


def _run(inputs):
    from concourse.bass_utils import run_bass_kernel_spmd

    in_maps, specs, TOT, bo_f, rown = _host_prep(
        inputs["x"], inputs["edge_index"], inputs["w1"], inputs["b1"],
        inputs["w2"], inputs["b2"], inputs["gw1"], inputs["gb1"],
        inputs["gw2"], inputs["gb2"], inputs["wo"], inputs["bo"])

    key = (hash(specs), TOT)
    if key not in _cache:
        _cache[key] = _build(specs, TOT, bo_f)
    nc = _cache[key]

    res = run_bass_kernel_spmd(nc, in_maps, list(range(8)))
    Yall = np.zeros((NP, B), dtype=np.float32)
    for k in range(NCORES):
        y_nb = res.results[k]["y"]          # [128, 4*NBC]
        for lb in range(NBC):
            lo = k * NSH + lb * 128
            for s in range(B):
                Yall[lo:lo + 128, s] = y_nb[:, lb * 4 + s]
    return Yall[rown, :].T.copy()


def kernel(**inputs):
    return _run(inputs)


# revision 40
# speedup vs baseline: 1.1082x; 1.0062x over previous
"""GraphWaveNet kernel for Trainium2 (Bass/Tile), 8 NeuronCores.

Design: edge sharding by SOURCE block, dense slot packing, and TWO
independent slice-pair pipelines (A = batch slices 0,1; B = 2,3) so the
four per-pipeline ReduceScatters interleave with each other's compute.

- Only t=11 survives the final 1x1 conv and the GCN doesn't mix time, so
  the conv stack is evaluated at t in {10,11} only and the GCN runs on
  B=4 slices packed as 256 columns (4 slices x 64 feats); pipeline A
  owns cols 0:128, B owns 128:256. The slices never mix, so A and B are
  fully independent after the conv -- their collectives pipeline:
    g0 A0 B0 RS0A [RS0B | fin0A g1a A1] RS1A [fin0B g1b B1] RS1B [fin1A] fin1B
- GCN identity: with Hs = dsq*h, agg_n = dsq_n * (sum_{e->n} Hs[src_e]
  + Hs[n]), then @W + b + relu.
- Sharding: core k owns node rows [1280k, 1280(k+1)). Edges live on the
  core owning their SRC, so gathers (dma_gather) read only the local
  table. Dense slot packing: block b's edges sit at global positions
  [S[b], S[b]+maxcnt[b]) where maxcnt = max over cores (SPMD-uniform
  schedule); specs are the 128-boundary pieces, each one one-hot P
  matmul into the block's PSUM accumulator (5-block group tiles, copy +
  DMA per group) forming a bf16 partial aggregate over all 10240 rows.
  One ReduceScatter per (layer, pipeline) returns each core its rows.
- PSUM chains: specs are emitted in position (= block-major) order, so
  a chain closes before the next block's opens -- never two open chains
  in one bank.
- Layer 0 gathers the full 256-col table once; both pipelines' scatter
  matmuls slice the same gathered tiles. Layer 1 tables are per-pipeline
  (128 cols), gathered separately with the same slot positions (eidx1).
- HBM layouts are chosen for >=512B contiguous DMA runs: aggN group
  regions are (p-major, q-minor) so each partition writes 1280B; table1
  rows are permuted (per 256-node bp: row = p*2 + lb) so finish writes
  512B runs; eidx1 bakes the permutation into the gather indices.
- dma_gather ucode contract (queue 0): flat index j of an instruction
  lands at out[j%128, j//128] and is read from idx tile position
  [16 + j%16, j//16] (int16).
"""

import sys

sys.path.insert(0, "/opt/trn_rl_repo")

import numpy as np
import ml_dtypes

B, T, N, FIN, H, E = 4, 12, 10000, 2, 64, 80000
NCORES = 8
NB80 = 80                 # dst blocks of 128 nodes
NP = NB80 * 128           # padded node count (10240)
NSH = NP // NCORES        # node rows per core (1280)
NBC = NB80 // NCORES      # node blocks per core (10)
D = 4 * H                 # 256 = 4 slices x 64 feats
DH = D // 2               # 128 = one pipeline's cols
SPG = 8                   # slots (of 128 edges) per dma_gather (1024 idxs)
NIG = SPG * 128           # indices per full gather instruction
ICPG = NIG // 16          # idx tile columns per full gather instr (64)

_cache = {}


def _balance(src, dst):
    """Node -> row permutation so every (core, dst-block) edge count <= 128.
    Phase 1: swap nodes between cores until pairwise core->core edge counts
    fit 10 blocks x 128. Phase 2: per-core greedy min-max packing into 10
    blocks with a repair pass. Heuristic: any residual overflow is handled
    by the dense spec packing (extra matmul pieces), not a correctness issue.
    """
    NPC = N // NCORES + (1 if N % NCORES else 0)   # 1250
    kc = np.minimum(np.arange(N) // NPC, NCORES - 1)
    out_deg = np.bincount(src, minlength=N)

    def mkT(kc):
        T = np.zeros((NCORES, NCORES), np.int64)
        np.add.at(T, (kc[src], kc[dst]), 1)
        return T

    T = mkT(kc)
    for _ in range(400):
        k, c = np.unravel_index(np.argmax(T), T.shape)
        if T[k, c] <= 1270:
            break
        ink = np.bincount(dst[kc[src] == k], minlength=N)
        nodes_c = np.flatnonzero(kc == c)
        n = nodes_c[np.argmax(ink[nodes_c] - 0.1 * out_deg[nodes_c])]
        c2 = np.argmin(T[k] + np.where(np.arange(NCORES) == c, 10 ** 9, 0))
        nodes_c2 = np.flatnonzero(kc == c2)
        n2 = nodes_c2[np.argmin(ink[nodes_c2] + 0.1 * out_deg[nodes_c2])]
        kc[n], kc[n2] = c2, c
        T = mkT(kc)

    dvec = np.zeros((N, NCORES), np.int64)
    np.add.at(dvec, (dst, kc[src]), 1)
    row_of_node = np.full(N, -1, np.int64)
    for c in range(NCORES):
        nodes = np.flatnonzero(kc == c)
        vv = dvec[nodes]
        order = np.argsort(-vv.sum(1), kind="stable")
        load = np.zeros((NBC, NCORES), np.int64)
        nn = np.zeros(NBC, np.int64)
        assign = np.zeros(len(nodes), np.int64)
        for i in order:
            v = vv[i]
            cand = np.flatnonzero(nn < 128)
            newmax = (load[cand] + v).max(axis=1)
            ok = newmax <= 128
            if ok.any():
                c2_ = cand[ok]
                b = c2_[np.argmin((load[c2_] + v).max(axis=1) * 1000
                                  + nn[c2_])]
            else:
                b = cand[np.argmin(newmax)]
            assign[i] = b
            load[b] += v
            nn[b] += 1
        for _ in range(300):
            viol = np.argwhere(load > 128)
            if len(viol) == 0:
                break
            b, k = viol[np.argmax(load[viol[:, 0], viol[:, 1]])]
            members = np.flatnonzero(assign == b)
            cand_n = members[vv[members, k] > 0]
            cand_n = cand_n[np.argsort(-vv[cand_n, k])]
            done = False
            for i in cand_n[:20]:
                v = vv[i]
                tgt = np.flatnonzero((nn < 128)
                                     & ((load + v) <= 128).all(axis=1))
                tgt = tgt[tgt != b]
                if len(tgt):
                    t = tgt[np.argmin((load[tgt] + v).max(axis=1))]
                    assign[i] = t
                    load[b] -= v
                    load[t] += v
                    nn[b] -= 1
                    nn[t] += 1
                    done = True
                    break
            if not done:
                break
        for b in range(NBC):
            sel = nodes[assign == b]
            base = c * NSH + b * 128
            row_of_node[sel] = base + np.arange(len(sel))
    return row_of_node


def _host_prep(x, edge_index, w1, b1, w2, b2, gw1, gb1, gw2, gb2, wo, bo):
    x = np.asarray(x, np.float32)
    src0 = np.asarray(edge_index[0]).astype(np.int64)
    dst0 = np.asarray(edge_index[1]).astype(np.int64)
    rown = _balance(src0, dst0)
    src, dst = rown[src0], rown[dst0]

    deg = np.bincount(dst0, minlength=N).astype(np.float64) + 1.0
    dsq = (deg ** -0.5).astype(np.float32)
    dsq_pad = np.ones(NP, dtype=np.float32)
    dsq_pad[rown] = dsq

    # ---- per-core edge partition by src owner, dst-sorted
    owner = src // NSH
    es_k, ed_k, cnt = [], [], np.zeros((NCORES, NB80), np.int64)
    for k in range(NCORES):
        m = owner == k
        es, ed = src[m], dst[m]
        o = np.argsort(ed, kind="stable")
        es_k.append(es[o])
        ed_k.append(ed[o])
        cnt[k] = np.bincount(ed[o] // 128, minlength=NB80)

    # dense schedule: block b's edges at positions [S[b], S[b]+mc[b])
    mc = np.maximum(1, cnt.max(axis=0))          # SPMD-uniform per block
    S = np.zeros(NB80 + 1, np.int64)
    S[1:] = np.cumsum(mc)
    TOTE = int(S[NB80])
    TOT = (TOTE + 127) // 128                    # slots
    # specs: 128-boundary pieces of each block segment, in position order
    specs = []                                   # (block, slot, pbase, plen, first, last)
    for b in range(NB80):
        a, e = int(S[b]), int(S[b] + mc[b])
        p = a
        while p < e:
            q = min(e, (p // 128 + 1) * 128)
            specs.append((b, p // 128, p % 128, q - p, p == a, q == e))
            p = q
    NMM = len(specs)

    # eidx / eidx1 / P (per-core data; schedule above is uniform)
    NGI = (TOT + SPG - 1) // SPG
    COLS = NGI * ICPG
    eidx_all = np.zeros((NCORES, 128, COLS), np.int16)
    eidx1_all = np.zeros((NCORES, 128, COLS), np.int16)
    P_all = np.zeros((NCORES, 128, NMM * 128), np.float32)
    for k in range(NCORES):
        es, ed = es_k[k], ed_k[k]
        bounds = np.searchsorted(ed, np.arange(NB80 + 1) * 128)
        for b in range(NB80):
            e0, e1 = int(bounds[b]), int(bounds[b + 1])
            ne = e1 - e0
            if ne == 0:
                continue
            jj = int(S[b]) + np.arange(ne)       # global positions
            rows = 16 + (jj % 16)
            cols = (jj // NIG) * ICPG + (jj % NIG) // 16
            loc = (es[e0:e1] - k * NSH).astype(np.int64)
            eidx_all[k, rows, cols] = loc.astype(np.int16)
            # table1 physical row: per 256-node bp, row = p*2 + lb
            lbg, p = loc // 128, loc % 128
            phys = (lbg // 2) * 256 + p * 2 + (lbg % 2)
            eidx1_all[k, rows, cols] = phys.astype(np.int16)
        for mmi, (b, slot, pbase, plen, first, last) in enumerate(specs):
            gpos = slot * 128 + pbase            # global position of spec start
            e0 = int(bounds[b]) + (gpos - int(S[b]))
            ne = min(plen, int(bounds[b + 1]) - e0)
            if ne > 0:
                P_all[k, pbase + np.arange(ne),
                      mmi * 128 + (ed[e0:e0 + ne] - b * 128)] = 1.0
    P_all = P_all.astype(ml_dtypes.float8_e4m3fn)   # one-hot: 1.0 exact in fp8

    # ---- conv input: per block 8 rows (t,c) for t in {9,10,11} + 2 zero rows,
    # cols = 4 slices x 128 nodes
    xpad = np.zeros((B, 3, FIN, NP), np.float32)
    xpad[:, :, :, rown] = x[:, 9:12, :, :].transpose(0, 1, 3, 2)  # [s, ti, c, n]
    xv = xpad.reshape(B, 6, NCORES, NBC, 128)                   # [s, row, k, blk, p]
    xt_all = np.zeros((NCORES, 8, NBC * 4 * 128), np.float32)
    xt_all[:, :6] = xv.transpose(2, 1, 3, 0, 4).reshape(NCORES, 6, NBC * 4 * 128)
    xt_all = xt_all.astype(ml_dtypes.bfloat16)

    dsqk_all = dsq_pad.reshape(NCORES, NBC, 128).transpose(0, 2, 1).copy()

    # ---- weights
    W1m = np.zeros((6, 64), np.float32)
    for kk in range(3):
        for c in range(FIN):
            W1m[2 * kk + c, :] = w1[:, c, 0, kk]
    W1ab = np.zeros((8, 128), np.float32)
    W1ab[0:6, 0:64] = W1m          # A: t10 (taps t9,t10,t11)
    W1ab[2:8, 64:128] = W1m        # B: t11 (taps t10,t11,t12=pad)
    W1ab = W1ab.astype(ml_dtypes.bfloat16)

    W2m = np.zeros((128, 64), np.float32)
    W2m[:64, :] = w2[:, :, 0, 0].T
    W2m[64:, :] = w2[:, :, 0, 1].T
    W2m = W2m.astype(ml_dtypes.bfloat16)

    b1s = np.concatenate([b1, b1]).reshape(128, 1).astype(np.float32)
    b2c = np.asarray(b2, np.float32).reshape(64, 1)
    gb1s = np.concatenate([gb1, gb1]).reshape(128, 1).astype(np.float32)
    gb2s = np.concatenate([gb2, gb2]).reshape(128, 1).astype(np.float32)
    gwd1 = np.zeros((128, 128), np.float32)
    gwd1[0:64, 0:64] = gw1
    gwd1[64:128, 64:128] = gw1
    gwd1 = gwd1.astype(ml_dtypes.bfloat16)
    gwd2 = np.zeros((128, 128), np.float32)
    gwd2[0:64, 0:64] = gw2
    gwd2[64:128, 64:128] = gw2
    gwd2 = gwd2.astype(ml_dtypes.bfloat16)
    wov = np.asarray(wo, np.float32)[0, :, 0, 0]
    wod2 = np.zeros((128, 2), np.float32)
    wod2[0:64, 0] = wov
    wod2[64:128, 1] = wov
    wod2 = wod2.astype(ml_dtypes.bfloat16)

    # pack all small weights into two tensors (one DMA each)
    cwb = np.zeros((128, 450), ml_dtypes.bfloat16)
    cwb[0:8, 0:128] = W1ab
    cwb[:, 128:192] = W2m
    cwb[:, 192:320] = gwd1
    cwb[:, 320:448] = gwd2
    cwb[:, 448:450] = wod2
    cwf = np.zeros((128, 14), np.float32)
    cwf[:, 0:1] = b1s
    cwf[0:64, 1:2] = b2c
    cwf[:, 2:3] = gb1s
    cwf[:, 3:4] = gb2s

    shared = {"cwb": cwb, "cwf": cwf}
    in_maps = []
    for k in range(NCORES):
        m = dict(shared)
        cf = m["cwf"].copy()
        cf[:, 4:14] = dsqk_all[k]
        m["cwf"] = cf
        m["xt"] = xt_all[k]
        m["eidx"] = eidx_all[k]
        m["eidx1"] = eidx1_all[k]
        m["P"] = P_all[k]
        in_maps.append(m)
    return (in_maps, tuple(specs), TOT,
            float(np.asarray(bo).reshape(-1)[0]), rown)


def _build(specs, TOT, bo_f):
    from concourse import bass, bacc, tile
    from concourse.masks import make_identity
    import mybir

    f32, bf16, i16 = mybir.dt.float32, mybir.dt.bfloat16, mybir.dt.int16
    NMM = len(specs)
    NGI = (TOT + SPG - 1) // SPG
    COLS = NGI * ICPG

    nc = bacc.Bacc("TRN2", target_bir_lowering=False, debug=False, num_devices=8,
                   dynamic_dma_scratch_size=65536)

    ext = {}
    for name, shape, dt in [
        ("xt", [8, NBC * 512], bf16), ("cwb", [128, 450], bf16),
        ("cwf", [128, 14], f32),
        ("eidx", [128, COLS], i16), ("eidx1", [128, COLS], i16),
        ("P", [128, NMM * 128], mybir.dt.float8e4),
    ]:
        ext[name] = nc.dram_tensor(name, shape, dt, kind="ExternalInput").ap()
    y_ext = nc.dram_tensor("y", [128, 4 * NBC], f32, kind="ExternalOutput").ap()
    table0 = nc.dram_tensor("table0", [NSH, D], bf16).ap()
    table1 = [nc.dram_tensor(f"table1{p}", [NSH, DH], bf16).ap() for p in range(2)]
    aggN = [[nc.dram_tensor(f"aggN{L}{p}", [NP, DH], bf16).ap() for p in range(2)]
            for L in range(2)]
    aggS = [[nc.dram_tensor(f"aggS{L}{p}", [NSH, DH], bf16).ap() for p in range(2)]
            for L in range(2)]

    RG = [list(range(NCORES))]

    with tile.TileContext(nc) as tc:
        with tc.tile_pool(name="const", bufs=1) as cp, \
             tc.tile_pool(name="hs", bufs=1) as hp, \
             tc.tile_pool(name="cv", bufs=3) as vp, \
             tc.tile_pool(name="xtp", bufs=1) as xp, \
             tc.tile_pool(name="g0", bufs=NGI) as gp0, \
             tc.tile_pool(name="g1", bufs=NGI + 2) as gp1, \
             tc.tile_pool(name="st", bufs=4) as sp, \
             tc.tile_pool(name="fv", bufs=4) as fv, \
             tc.tile_pool(name="fa", bufs=12) as fa:
            cwb = cp.tile([128, 450], bf16, tag="cwb")
            nc.sync.dma_start(cwb[:], ext["cwb"][:])
            cwf = cp.tile([128, 14], f32, tag="cwf")
            nc.sync.dma_start(cwf[:], ext["cwf"][:])
            ct = {
                "W1ab": cwb[0:8, 0:128], "W2m": cwb[:, 128:192],
                "gwd1": cwb[:, 192:320], "gwd2": cwb[:, 320:448],
                "wod2": cwb[:, 448:450],
                "b1s": cwf[:, 0:1], "b2c": cwf[0:64, 1:2],
                "gb1s": cwf[:, 2:3], "gb2s": cwf[:, 3:4],
                "dsqk": cwf[:, 4:14],
            }
            late = {}
            for name in ("eidx", "eidx1"):
                lt = cp.tile(list(ext[name].shape), ext[name].dtype, tag=name,
                             name=name)
                late[name] = lt
                ct[name] = lt
            Pt = cp.tile(list(ext["P"].shape), ext["P"].dtype, tag="P")
            ct["P"] = Pt
            ident = cp.tile([128, 128], bf16, tag="ident")
            make_identity(nc, ident[:])
            dsqd = cp.tile([128, NBC * 128], bf16, tag="dsqd")
            y_nb = cp.tile([128, 4 * NBC], f32, tag="ynb")

            hs0 = hp.tile([128, NBC * D], bf16, tag="hs0")
            hs1 = [hp.tile([128, NBC * DH], bf16, tag=f"hs1{p}",
                           name=f"hs1{p}") for p in range(2)]

            # ---- conv stage: local table0 shard = dsq * relu(conv2(relu(conv1 x)))
            with tc.tile_pool(name="c1", bufs=2, space="PSUM") as p1, \
                 tc.tile_pool(name="c2", bufs=2, space="PSUM") as p2, \
                 tc.tile_pool(name="c3", bufs=2, space="PSUM") as p3:
                xts = xp.tile([8, NBC * 512], bf16, tag="xts")
                nc.sync.dma_start(xts[:], ext["xt"][:])
                for name in ("eidx", "eidx1"):
                    nc.sync.dma_start(late[name][:], ext[name][:])
                nc.sync.dma_start(Pt[:], ext["P"][:])
                for lb in range(NBC):
                    ph1 = p1.tile([128, 512], f32, tag="ph1", space="PSUM")
                    nc.tensor.matmul(ph1[:], lhsT=ct["W1ab"][:],
                                     rhs=xts[:, lb * 512:(lb + 1) * 512],
                                     start=True, stop=True)
                    h1 = vp.tile([128, 512], bf16, tag="h1")
                    nc.vector.tensor_scalar(h1[:, 0:256], ph1[:, 0:256],
                                            ct["b1s"][:, 0:1], 0.0,
                                            mybir.AluOpType.add,
                                            mybir.AluOpType.max)
                    nc.scalar.activation(h1[:, 256:512], ph1[:, 256:512],
                                         mybir.ActivationFunctionType.Relu,
                                         bias=ct["b1s"][:, 0:1])
                    ph2 = p2.tile([64, 512], f32, tag="ph2", space="PSUM")
                    nc.tensor.matmul(ph2[:], lhsT=ct["W2m"][:], rhs=h1[:],
                                     start=True, stop=True)
                    h2 = vp.tile([64, 512], bf16, tag="h2")
                    nc.scalar.activation(h2[:, 0:256], ph2[:, 0:256],
                                         mybir.ActivationFunctionType.Relu,
                                         bias=ct["b2c"][:, 0:1])
                    nc.vector.tensor_scalar(h2[:, 256:512], ph2[:, 256:512],
                                            ct["b2c"][:, 0:1], 0.0,
                                            mybir.AluOpType.add,
                                            mybir.AluOpType.max)
                    ptp = p3.tile([128, 256], bf16, tag="ptp", space="PSUM")
                    for s in range(4):
                        nc.tensor.transpose(ptp[:, 64 * s:64 * (s + 1)],
                                            h2[:, s * 128:(s + 1) * 128],
                                            ident[0:64, 0:64])
                    if lb % 2 == 0:
                        nc.vector.tensor_scalar_mul(
                            hs0[:, lb * D:(lb + 1) * D], ptp[:],
                            ct["dsqk"][:, lb:lb + 1])
                    else:
                        nc.scalar.activation(
                            hs0[:, lb * D:(lb + 1) * D], ptp[:],
                            mybir.ActivationFunctionType.Copy,
                            scale=ct["dsqk"][:, lb:lb + 1])
                nc.sync.dma_start(
                    table0.rearrange("(lb p) f -> p lb f", p=128), hs0[:])
                for lb in range(NBC):
                    nc.vector.tensor_scalar_mul(
                        dsqd[:, lb * 128:(lb + 1) * 128], ident[:],
                        ct["dsqk"][:, lb:lb + 1])

            # ---- main pipeline PSUM pools (alive through both layers)
            with tc.tile_pool(name="sc", bufs=2, space="PSUM") as qp, \
                 tc.tile_pool(name="ps", bufs=3, space="PSUM") as ftp, \
                 tc.tile_pool(name="fb", bufs=1, space="PSUM") as ftb:
                fwp = ftp

                def emit_gathers(gp, tbl, width, idxt):
                    gts = []
                    for gi in range(NGI):
                        nsl = min(SPG, TOT - gi * SPG)
                        g = gp.tile([128, SPG, width], bf16, tag="g",
                                    name=f"g{gi}")
                        nc.gpsimd.dma_gather(
                            g[:, 0:nsl, :], tbl[:],
                            idxt[:, gi * ICPG:gi * ICPG + nsl * 8],
                            nsl * 128, nsl * 128, width)
                        gts.append(g)
                    return gts

                def emit_scatter(gts, c0, c1v, aggN_ts, par):
                    """One-hot scatter matmuls over all specs. When c1v-c0 is
                    256 both pipelines' halves are produced by one matmul and
                    staged to the two aggN tensors in aggN_ts."""
                    wid = c1v - c0
                    pb = None
                    for mmi, (b, slot, pbase, plen, first, last) in \
                            enumerate(specs):
                        g = gts[slot // SPG]
                        w = b % 5
                        if w == 0 and first:
                            pb = qp.tile([128, 5, wid], f32, tag="pb",
                                         space="PSUM", name="pb")
                        nc.tensor.matmul(
                            pb[:, w, :],
                            lhsT=ct["P"][:, mmi * 128:(mmi + 1) * 128],
                            rhs=g[:, slot % SPG, c0:c1v],
                            start=first, stop=last)
                        if last and w == 4:
                            # staging copies stay on DVE: the finish stages
                            # run concurrently on Act, so the aggN writes
                            # that gate the next collective never queue
                            # behind finish element-ops
                            grp = b // 5
                            for hi, aggN_t in enumerate(aggN_ts):
                                stg = sp.tile([128, 5, DH], bf16, tag="stg")
                                if par == 0:
                                    nc.vector.tensor_copy(
                                        stg[:],
                                        pb[:, :, hi * DH:hi * DH + DH])
                                else:
                                    nc.scalar.activation(
                                        stg[:],
                                        pb[:, :, hi * DH:hi * DH + DH],
                                        mybir.ActivationFunctionType.Copy)
                                nc.sync.dma_start(
                                    aggN_t[grp * 640:(grp + 1) * 640, :]
                                    .rearrange("(p q) f -> p q f", p=128),
                                    stg[:])

                def emit_finish(L, p):
                    """Self-loop + dsq + W + bias + relu for pipeline p."""
                    gwd = ct["gwd1"] if L == 0 else ct["gwd2"]
                    gbs = ct["gb1s"] if L == 0 else ct["gb2s"]
                    hs_cur = hs0 if L == 0 else hs1[p]
                    agg = aggS[L][p]
                    asbs = []
                    for pi in range(NBC // 2):
                        asb = fa.tile([128, 2, DH], bf16, tag="asb",
                                      name=f"asb{L}{p}{pi}")
                        b0 = 2 * pi
                        g0_, q0 = divmod(b0, 5)
                        g1_, q1 = divmod(b0 + 1, 5)
                        if g0_ == g1_:
                            nc.sync.dma_start(
                                asb[:],
                                agg[g0_ * 640:(g0_ + 1) * 640, :]
                                .rearrange("(p q) f -> p q f", p=128)
                                [:, q0:q0 + 2, :])
                        else:
                            nc.sync.dma_start(
                                asb[:, 0:1, :],
                                agg[g0_ * 640:(g0_ + 1) * 640, :]
                                .rearrange("(p q) f -> p q f", p=128)
                                [:, q0:q0 + 1, :])
                            nc.sync.dma_start(
                                asb[:, 1:2, :],
                                agg[g1_ * 640:(g1_ + 1) * 640, :]
                                .rearrange("(p q) f -> p q f", p=128)
                                [:, q1:q1 + 1, :])
                        asbs.append(asb)
                    for pi in range(NBC // 2):
                        asb = asbs[pi]
                        tp2 = ftp.tile([128, 2 * DH], f32, tag="ps",
                                       space="PSUM", name="tp2")
                        for c in range(2):
                            b = 2 * pi + c
                            if L == 0:
                                hsl = hs_cur[:, b * D + p * DH:
                                             b * D + (p + 1) * DH]
                            else:
                                hsl = hs_cur[:, b * DH:(b + 1) * DH]
                            nc.tensor.matmul(
                                tp2[:, c * DH:(c + 1) * DH],
                                lhsT=asb[:, c, :],
                                rhs=dsqd[:, b * 128:(b + 1) * 128],
                                start=True, stop=False)
                            nc.tensor.matmul(
                                tp2[:, c * DH:(c + 1) * DH],
                                lhsT=hsl,
                                rhs=dsqd[:, b * 128:(b + 1) * 128],
                                start=False, stop=True)
                        tps = fv.tile([128, 2 * DH], bf16, tag="tps")
                        nc.scalar.activation(
                            tps[:], tp2[:],
                            mybir.ActivationFunctionType.Copy)
                        wp2 = fwp.tile([128, 2 * DH], f32, tag="ps",
                                       space="PSUM", name="wp2")
                        nc.tensor.matmul(wp2[:], lhsT=gwd[:], rhs=tps[:],
                                         start=True, stop=True)
                        h42 = fv.tile([128, 2 * DH], bf16, tag="h42")
                        nc.scalar.activation(h42[:], wp2[:],
                                             mybir.ActivationFunctionType.Relu,
                                             bias=gbs[:, 0:1])
                        if L == 0:
                            tb2 = ftb.tile([128, 2, 128], bf16, tag="fb",
                                           space="PSUM", name="tb2")
                            for c in range(2):
                                nc.tensor.transpose(
                                    tb2[:, c, :],
                                    h42[:, c * 128:(c + 1) * 128], ident[:])
                            hsn = hs1[p]
                            for c in range(2):
                                b = 2 * pi + c
                                nc.scalar.activation(
                                    hsn[:, b * DH:(b + 1) * DH],
                                    tb2[:, c, :],
                                    mybir.ActivationFunctionType.Copy,
                                    scale=ct["dsqk"][:, b:b + 1])
                            # permuted rows: p*2 + lb within the bp
                            nc.sync.dma_start(
                                table1[p][pi * 256:(pi + 1) * 256, :]
                                .rearrange("(p lb) f -> p (lb f)", p=128),
                                hsn[:, pi * 2 * DH:(pi + 1) * 2 * DH])
                        else:
                            yp2 = ftb.tile([128, 4], f32, tag="fb",
                                           space="PSUM", name="yp2")
                            for c in range(2):
                                nc.tensor.matmul(
                                    yp2[:, c * 2:(c + 1) * 2],
                                    lhsT=h42[:, c * 128:(c + 1) * 128],
                                    rhs=ct["wod2"][:],
                                    start=True, stop=True)
                            for c in range(2):
                                b = 2 * pi + c
                                nc.vector.tensor_scalar_add(
                                    y_nb[:, 4 * b + 2 * p:4 * b + 2 * p + 2],
                                    yp2[:, c * 2:(c + 1) * 2], bo_f)

                # ---- pipeline schedule
                gts0 = emit_gathers(gp0, table0, D, ct["eidx"])
                emit_scatter(gts0, 0, DH, [aggN[0][0]], 0)
                emit_scatter(gts0, DH, D, [aggN[0][1]], 1)
                nc.gpsimd.collective_compute(
                    "ReduceScatter", mybir.AluOpType.add, replica_groups=RG,
                    ins=[aggN[0][0][:]], outs=[aggS[0][0][:]])
                nc.gpsimd.collective_compute(
                    "ReduceScatter", mybir.AluOpType.add, replica_groups=RG,
                    ins=[aggN[0][1][:]], outs=[aggS[0][1][:]])
                emit_finish(0, 0)
                gts1a = emit_gathers(gp1, table1[0], DH, ct["eidx1"])
                emit_scatter(gts1a, 0, DH, [aggN[1][0]], 0)
                emit_finish(0, 1)
                nc.gpsimd.collective_compute(
                    "ReduceScatter", mybir.AluOpType.add, replica_groups=RG,
                    ins=[aggN[1][0][:]], outs=[aggS[1][0][:]])
                gts1b = emit_gathers(gp1, table1[1], DH, ct["eidx1"])
                emit_scatter(gts1b, 0, DH, [aggN[1][1]], 1)
                nc.gpsimd.collective_compute(
                    "ReduceScatter", mybir.AluOpType.add, replica_groups=RG,
                    ins=[aggN[1][1][:]], outs=[aggS[1][1][:]])
                emit_finish(1, 0)
                emit_finish(1, 1)

            nc.sync.dma_start(y_ext[:], y_nb[:])
    nc.compile()
    return nc


def _run(inputs):
    from concourse.bass_utils import run_bass_kernel_spmd

    in_maps, specs, TOT, bo_f, rown = _host_prep(
        inputs["x"], inputs["edge_index"], inputs["w1"], inputs["b1"],
        inputs["w2"], inputs["b2"], inputs["gw1"], inputs["gb1"],
        inputs["gw2"], inputs["gb2"], inputs["wo"], inputs["bo"])

    key = (hash(specs), TOT)
    if key not in _cache:
        _cache[key] = _build(specs, TOT, bo_f)
    nc = _cache[key]

    res = run_bass_kernel_spmd(nc, in_maps, list(range(8)))
    Yall = np.zeros((NP, B), dtype=np.float32)
    for k in range(NCORES):
        y_nb = res.results[k]["y"]          # [128, 4*NBC]
        for lb in range(NBC):
            lo = k * NSH + lb * 128
            for s in range(B):
                Yall[lo:lo + 128, s] = y_nb[:, lb * 4 + s]
    return Yall[rown, :].T.copy()


def kernel(**inputs):
    return _run(inputs)
